# revision 2
# baseline (speedup 1.0000x reference)
"""DTranNER CRF loss kernel for Trainium2 (8 NeuronCores, data-parallel over batch).

v2 redesign vs baseline:
  * 128-partition layout: (i, b) with i = 4 k-blocks of 6 states, b = 32
    sentences  ->  DVE mult [128,144] (2x bf16), ACT exp at full width.
  * bf16 HBM stream (host casts fpp/feats to bf16): halves DMA bytes.
  * No renorm anywhere: exp pre-scales (CP pairwise, CU+CT unary with
    CU+CT = 1+ln 24) keep per-step growth ~1; fp32/bf16 exponent range
    absorbs the +-4-sigma drift of 256-step half-chains.
  * PSUM->SBUF state copies split across ACT (fwd) / DVE (bwd) to balance
    engine busy; unary state stays in PSUM (read directly by the DVE mult).
"""

import numpy as np
import ml_dtypes
from contextlib import ExitStack

import concourse.bass as bass
import concourse.bacc as bacc
import concourse.tile as tile
from concourse import mybir
from concourse.bass_utils import run_bass_kernel_spmd

FP = mybir.dt.float32
BF = mybir.dt.bfloat16

B, T, K = 256, 512, 24
START, STOP = 22, 23
NCORES = 8
NS, KB = 4, 6          # K = NS*KB k-block split
P = NS * 32            # 128 partitions (i-major: p = i*32 + b)
UROW = 64

CP = 3.678            # pairwise exp pre-scale = ln(24)+0.5 (zero mean drift)
CU = 2.0              # unary emission exp pre-scale
CT = 2.0374           # unary transition pre-scale (CU+CT = measured natural rate)

AF = mybir.ActivationFunctionType
ALU = mybir.AluOpType
AX = mybir.AxisListType


def build_kernel(BC=32, TT=512, TC=20, CHOP=2, FIRST=8, SBUFS=3, BIGB=2, EBIGB=3,
                 FCOPY="dve", BCOPY="dve", JOINT=0, PAIRCOPY="dve", UCOPY="psum",
                 PSB=2, PSBF=None):
    assert BC == 32
    NF2 = K * KB       # 144
    H = TT // 2        # fwd pairwise steps (matrices t = 0..H-1)
    HB = TT - 1 - H    # bwd steps (matrices t = TT-2..H, transposed)
    SL = H             # unary slots

    nc = bacc.Bacc("TRN2", target_bir_lowering=False)
    fppF = nc.dram_tensor("fppF", [NS, BC, H, NF2], BF, kind="ExternalInput")
    fppB = nc.dram_tensor("fppB", [NS, BC, HB, NF2], BF, kind="ExternalInput")
    winit = nc.dram_tensor("winit", [P, KB], FP, kind="ExternalInput")
    ftp2 = nc.dram_tensor("ftp2", [SL, UROW, BC], BF, kind="ExternalInput")
    eflast = nc.dram_tensor("eflast", [K, BC], FP, kind="ExternalInput")
    transPK = nc.dram_tensor("transPK", [UROW, K], FP, kind="ExternalInput")
    gvals = nc.dram_tensor("gvals", [BC, 3 * TT + 4], FP, kind="ExternalInput")
    selpack = nc.dram_tensor("selpack", [P, 32 + P + K], BF, kind="ExternalInput")
    nll = nc.dram_tensor("nll", [BC], FP, kind="ExternalOutput")

    with tile.TileContext(nc) as tc, ExitStack() as ctx:
        sb = ctx.enter_context(tc.tile_pool(name="sb", bufs=SBUFS))
        big = ctx.enter_context(tc.tile_pool(name="big", bufs=BIGB))
        ebig = ctx.enter_context(tc.tile_pool(name="ebig", bufs=EBIGB))
        per = ctx.enter_context(tc.tile_pool(name="per", bufs=1))
        psF = ctx.enter_context(tc.tile_pool(name="psF", bufs=(PSBF or PSB), space="PSUM"))
        psB = ctx.enter_context(tc.tile_pool(name="psB", bufs=PSB, space="PSUM"))
        psU = ctx.enter_context(tc.tile_pool(name="psU", bufs=2, space="PSUM"))
        ps1 = ctx.enter_context(tc.tile_pool(name="ps1", bufs=1, space="PSUM"))

        # ---------------- constants ----------------
        cpb = per.tile([128, 1], FP, tag="cpb", name="cpb")
        nc.vector.memset(cpb[:], -CP)
        cub = per.tile([128, 1], FP, tag="cub", name="cub")
        nc.vector.memset(cub[:], -CU)
        ctb = per.tile([128, 1], FP, tag="ctb", name="ctb")
        nc.vector.memset(ctb[:], -CT)

        selpack_sb = per.tile([P, 32 + P + K], BF, tag="selpack", name="selpack_sb")
        nc.sync.dma_start(out=selpack_sb[:], in_=selpack[:])
        sel32_sb = selpack_sb[:, 0:32]
        sel4_sb = selpack_sb[:, 32 : 32 + P]
        selDN_sb = selpack_sb[0:UROW, 32 + P : 32 + P + K]

        # unary stationary weights (block matrix): uw = exp(transPK - CT)
        uwst = per.tile([UROW, K], FP, tag="uwst", name="uwst")
        nc.sync.dma_start(out=uwst[:], in_=transPK[:])
        uw = per.tile([UROW, UROW], BF, tag="uw", name="uw")
        nc.vector.memset(uw[:], 0.0)
        nc.scalar.activation(out=uw[0:K, 0:K], in_=uwst[0:K, :], func=AF.Exp,
                             bias=ctb[0:K, :])
        nc.scalar.activation(out=uw[32 : 32 + K, 32 : 32 + K], in_=uwst[32 : 32 + K, :],
                             func=AF.Exp, bias=ctb[0:K, :])

        # ---------------- unary Ef table ----------------
        eft = per.tile([UROW, SL * BC], BF, tag="eft", name="eft")
        nchunk = 8
        cs2 = SL // nchunk
        cstep = cs2 * BC
        src = ftp2[:, :, :].rearrange("s r j -> r s j")

        def load_eft_chunk(c):
            ftile = big.tile([UROW, cstep], BF, tag="ftp_in", name="ftile_u")
            nc.sync.dma_start(
                out=ftile[:].rearrange("p (s j) -> p s j", j=BC),
                in_=src[:, c * cs2 : (c + 1) * cs2, :],
            )
            nc.scalar.activation(
                out=eft[:, c * cstep : (c + 1) * cstep], in_=ftile[:],
                func=AF.Exp, bias=cub[0:UROW, :],
            )

        load_eft_chunk(0)

        # ---------------- state init ----------------
        # fwd pairwise state: e_START one-hot. START=22 -> i=3, kk=4.
        uf0 = per.tile([P, KB], BF, tag="uf0", name="uf0")
        nc.vector.memset(uf0[:], 0.0)
        nc.vector.memset(uf0[96:128, 4:5], 1.0)

        # bwd pairwise init: exp(fpp[b, T-1, STOP, :] - CP), sliced (i b) kk
        wf = sb.tile([P, KB], FP, tag="wf", name="wf")
        nc.sync.dma_start(out=wf[:], in_=winit[:, :])
        ub0 = per.tile([P, KB], BF, tag="ub0", name="ub0")
        nc.scalar.activation(out=ub0[:], in_=wf[:], func=AF.Exp, bias=cpb[0:P, :])

        # unary state [UROW, BC]
        us0 = per.tile([UROW, BC], BF, tag="us0", name="us0")
        nc.vector.memset(us0[:], 0.0)
        row1 = sb.tile([1, BC], BF, tag="row1", name="row1")
        nc.vector.memset(row1[:], 1.0)
        nc.sync.dma_start(out=us0[START : START + 1, :], in_=row1[:])
        tstop = sb.tile([UROW, 1], FP, tag="tstop", name="tstop")
        nc.sync.dma_start(
            out=tstop[32 : 32 + K, :],
            in_=transPK[32 + STOP : 32 + STOP + 1, :].rearrange("o k -> k o"),
        )
        tstop_e = sb.tile([UROW, 1], BF, tag="tstop_e", name="tstop_e")
        nc.scalar.activation(out=tstop_e[32 : 32 + K, :], in_=tstop[32 : 32 + K, :], func=AF.Exp)
        nc.vector.tensor_copy(
            out=us0[32 : 32 + K, :], in_=tstop_e[32 : 32 + K, :].broadcast_to([K, BC])
        )

        # ---------------- helpers ----------------
        gv = per.tile([BC, 3 * TT + 4], FP, tag="gv", name="gv")
        sc_early = per.tile([BC, 1], FP, tag="sc_early", name="sc_early")

        def pair_mm(eX, m, st, ups, c0, tag, ptile):
            """Pairwise chain step: DVE mult + 24 accumulating PE matmuls into
            ups[:, c0:c0+KB].  `st` is a 2D [P, KB] AP (SBUF or PSUM).
            prod goes into ptile[:, m*NF2:(m+1)*NF2] (per-chunk tile, subtile
            deps -> no per-step WAW sem waits on DVE)."""
            e3 = eX[:, m * NF2 : (m + 1) * NF2].rearrange("q (a b) -> q a b", a=K)
            p3 = ptile[:, m * NF2 : (m + 1) * NF2].rearrange("q (a b) -> q a b", a=K)
            ub = st.unsqueeze(1).broadcast_to([P, K, KB])
            nc.vector.tensor_tensor(out=p3, in0=e3, in1=ub, op=ALU.mult)
            for ip in range(NS):
                tp = (0, ip * 32)
                for kk in range(KB):
                    rhs = p3[:, ip * KB : (ip + 1) * KB, kk]
                    nc.tensor.matmul(
                        out=ups[ip * 32 : (ip + 1) * 32, c0 : c0 + KB],
                        lhsT=sel32_sb, rhs=rhs,
                        start=(kk == 0), stop=(kk == KB - 1),
                        tile_position=tp,
                    )

        def chain_copy(ups, eng, tag):
            ns_ = sb.tile([P, KB], BF, tag=f"ns{tag}", name=f"ns{tag}")
            if eng == "act":
                nc.scalar.activation(out=ns_[:], in_=ups[:, 0:KB], func=AF.Copy)
            else:
                nc.vector.tensor_copy(out=ns_[:], in_=ups[:, 0:KB])
            return ns_

        def chain_copy_joint(ups, hadB):
            w = 2 * KB if hadB else KB
            ns_ = sb.tile([P, 2 * KB], BF, tag="nsJ", name="nsJ")
            nc.vector.tensor_copy(out=ns_[:, 0:w], in_=ups[:, 0:w])
            return ns_

        # ---------------- main streamed loop ----------------
        def exp_chunks(nt):
            cs = (nt + CHOP - 1) // CHOP if CHOP else nt
            return [(a, min(a + cs, nt)) for a in range(0, nt, cs)]

        plan = [0]
        t_acc = min(FIRST, H) if FIRST else min(TC, H)
        while t_acc < H:
            plan.append(t_acc)
            t_acc += min(TC, H - t_acc)
        stF, stB = uf0[:, :], ub0[:, :]
        pendF = pendB = pendJ = pendU = None
        stU = us0
        nU = 0
        for it, t0 in enumerate(plan):
            if 1 <= it <= nchunk - 1:
                load_eft_chunk(it)
            if it == nchunk:
                nc.sync.dma_start(out=gv[:], in_=gvals[:])
                nc.vector.tensor_reduce(out=sc_early[:], in_=gv[:], axis=AX.X, op=ALU.add)
            t_next = plan[it + 1] if it + 1 < len(plan) else H
            ntF = t_next - t0
            ntB = max(0, min(t_next, HB) - t0)
            ftileF = big.tile([P, TC * NF2], BF, tag="ftileF", name="ftileF")
            for c0, c1 in exp_chunks(ntF):
                nc.sync.dma_start(
                    out=ftileF[:, c0 * NF2 : c1 * NF2],
                    in_=fppF[:, :, t0 + c0 : t0 + c1, :].rearrange("i b t f -> (i b) (t f)"),
                )
            eF = ebig.tile([P, TC * NF2], BF, tag="eF", name="eF")
            for c0, c1 in exp_chunks(ntF):
                nc.scalar.activation(
                    out=eF[:, c0 * NF2 : c1 * NF2], in_=ftileF[:, c0 * NF2 : c1 * NF2],
                    func=AF.Exp, bias=cpb[0:P, :],
                )
            if ntB > 0:
                ftileB = big.tile([P, TC * NF2], BF, tag="ftileB", name="ftileB")
                for c0, c1 in exp_chunks(ntB):
                    nc.sync.dma_start(
                        out=ftileB[:, c0 * NF2 : c1 * NF2],
                        in_=fppB[:, :, t0 + c0 : t0 + c1, :].rearrange("i b t f -> (i b) (t f)"),
                    )
                eB = ebig.tile([P, TC * NF2], BF, tag="eB", name="eB")
                for c0, c1 in exp_chunks(ntB):
                    nc.scalar.activation(
                        out=eB[:, c0 * NF2 : c1 * NF2], in_=ftileB[:, c0 * NF2 : c1 * NF2],
                        func=AF.Exp, bias=cpb[0:P, :],
                    )

            prodF_t = big.tile([P, TC * NF2], BF, tag="prodF", name="prodF_t")
            prodB_t = big.tile([P, TC * NF2], BF, tag="prodB", name="prodB_t")
            usm_t = big.tile([UROW, TC * BC], BF, tag="usm_t", name="usm_t")
            for m in range(ntF):
                # ---- unary slot ----
                g = nU
                ef_sl = eft[:, g * BC : (g + 1) * BC]
                if pendU is not None:
                    stU_sb = sb.tile([UROW, BC], BF, tag="stU", name="stU_sb")
                    nc.scalar.activation(out=stU_sb[:], in_=pendU[:], func=AF.Copy)
                    stU = stU_sb
                    pendU = None
                usm = usm_t[:, m * BC : (m + 1) * BC]
                nc.vector.tensor_tensor(out=usm, in0=stU[:], in1=ef_sl, op=ALU.mult)
                nU += 1
                vu_ps = psU.tile([UROW, BC], FP, tag="vu", name="vu_ps")
                nc.tensor.matmul(out=vu_ps[:], lhsT=uw[:], rhs=usm, start=True, stop=True)
                if UCOPY == "act":
                    pendU = vu_ps
                else:
                    stU = vu_ps

                has_b = m < ntB
                if JOINT:
                    # one PSUM tile for both chains, one joint copy
                    if pendJ is not None:
                        stJ = chain_copy_joint(pendJ[0], pendJ[1])
                        stF = stJ[:][:, 0:KB]
                        if pendJ[1]:
                            stB = stJ[:][:, KB : 2 * KB]
                        pendJ = None
                    upsJ = psF.tile([P, 2 * KB], FP, tag="upsJ", name="upsJ")
                    pair_mm(eF, m, stF, upsJ, 0, "F", prodF_t)
                    if has_b:
                        pair_mm(eB, m, stB, upsJ, KB, "B", prodB_t)
                    pendJ = (upsJ, has_b)
                elif PAIRCOPY == "none":
                    # mults read PSUM state directly; no copies
                    upsF = psF.tile([P, KB], FP, tag="upsF", name="upsF")
                    pair_mm(eF, m, stF, upsF, 0, "F", prodF_t)
                    stF = upsF[:, :]
                    if has_b:
                        upsB = psB.tile([P, KB], FP, tag="upsB", name="upsB")
                        pair_mm(eB, m, stB, upsB, KB * 0, "B", prodB_t)
                        stB = upsB[:, :]
                else:
                    if pendF is not None:
                        stF = chain_copy(pendF, FCOPY, "F")[:, :]
                        pendF = None
                    upsF = psF.tile([P, KB], FP, tag="upsF", name="upsF")
                    pair_mm(eF, m, stF, upsF, 0, "F", prodF_t)
                    pendF = upsF

                    if pendB is not None:
                        stB = chain_copy(pendB, BCOPY, "B")[:, :]
                        pendB = None
                    if has_b:
                        upsB = psB.tile([P, KB], FP, tag="upsB", name="upsB")
                        pair_mm(eB, m, stB, upsB, 0, "B", prodB_t)
                        pendB = upsB

        # ---------------- flush final deferred copies ----------------
        if pendJ is not None:
            stJ = chain_copy_joint(pendJ[0], pendJ[1])
            stF = stJ[:][:, 0:KB]
            if pendJ[1]:
                stB = stJ[:][:, KB : 2 * KB]
            pendJ = None
        if pendF is not None:
            stF = chain_copy(pendF, FCOPY, "F")[:, :]
            pendF = None
        if pendB is not None:
            stB = chain_copy(pendB, BCOPY, "B")[:, :]
            pendB = None
        if pendU is not None:
            stU_sb = sb.tile([UROW, BC], BF, tag="stU", name="stU_sb")
            nc.scalar.activation(out=stU_sb[:], in_=pendU[:], func=AF.Copy)
            stU = stU_sb
            pendU = None
        if PAIRCOPY == "none":
            # final states live in PSUM; the meet mult may read only one
            # PSUM operand, so land stB in SBUF first
            stB_sb = sb.tile([P, KB], BF, tag="nsB", name="stB_fin")
            nc.vector.tensor_copy(out=stB_sb[:], in_=stB)
            stB = stB_sb[:, :]

        # ---------------- tails ----------------
        # pairwise meet: q[b] = sum_k uF[b,k]*uB[b,k]
        qm = sb.tile([P, KB], BF, tag="qm", name="qm")
        nc.vector.tensor_tensor(out=qm[:], in0=stF, in1=stB, op=ALU.mult)
        qr = sb.tile([P, 1], BF, tag="qr", name="qr")
        with nc.allow_low_precision("meet partial"):
            nc.vector.tensor_reduce(out=qr[:], in_=qm[:], axis=AX.X, op=ALU.add)
        qrep = ps1.tile([P, 1], FP, tag="pmisc", name="qrep")
        nc.tensor.matmul(out=qrep[:], lhsT=sel4_sb, rhs=qr[:], start=True, stop=True)
        lq = sb.tile([P, 1], FP, tag="lq", name="lq")
        nc.scalar.activation(out=lq[:], in_=qrep[:], func=AF.Ln)

        # unary tail: all per-b results assembled partition-major [32, 1]
        efl = sb.tile([K, BC], FP, tag="efl", name="efl")
        nc.sync.dma_start(out=efl[:], in_=eflast[:])
        efl_e = sb.tile([K, BC], BF, tag="efl_e", name="efl_e")
        nc.scalar.activation(out=efl_e[:], in_=efl[:], func=AF.Exp)
        ustail = sb.tile([UROW, BC], BF, tag="ustail", name="ustail")
        nc.scalar.activation(out=ustail[:], in_=stU[:], func=AF.Copy)
        # bring bwd rows 32..32+K down to partitions 0..K via selector matmul
        usb_ps = ps1.tile([K, BC], FP, tag="pmisc", name="usb_ps")
        nc.tensor.matmul(out=usb_ps[:], lhsT=selDN_sb, rhs=ustail[:], start=True, stop=True)
        um = sb.tile([K, BC], BF, tag="um", name="um")
        nc.vector.tensor_tensor(out=um[:], in0=ustail[0:K, :], in1=efl_e[:], op=ALU.mult)
        nc.vector.tensor_tensor(out=um[:], in0=um[:], in1=usb_ps[:], op=ALU.mult)
        ones_k = sb.tile([K, 1], BF, tag="ones_k", name="ones_k")
        nc.vector.memset(ones_k[:], 1.0)
        au_t = ps1.tile([32, 1], FP, tag="pmisc", name="au_t")
        nc.tensor.matmul(out=au_t[:], lhsT=um[:], rhs=ones_k[:], start=True, stop=True)
        lau_t = sb.tile([32, 1], FP, tag="lau_t", name="lau_t")
        nc.scalar.activation(out=lau_t[:], in_=au_t[:], func=AF.Ln)

        # score reduction + final combine (all [32, 1] partition-major)
        CONST = CP * (H + HB + 1) + (CU + CT) * (2 * SL)
        res = sb.tile([BC, 1], FP, tag="res", name="res")
        nc.vector.tensor_tensor(out=res[:], in0=lq[0:32, :], in1=lau_t[:], op=ALU.add)
        nc.vector.tensor_scalar(out=res[:], in0=res[:], scalar1=CONST, scalar2=None, op0=ALU.add)
        nc.vector.tensor_tensor(out=res[:], in0=res[:], in1=sc_early[:], op=ALU.subtract)
        nc.sync.dma_start(out=nll[:], in_=res[:].rearrange("b o -> (b o)"))

    nc.compile()
    return nc


# ======================= host-side prep =======================

def prep_core_inputs(feats, fpp, transitions, tags, b0, BC, TT):
    H = TT // 2
    HB = TT - 1 - H
    fe = feats[b0 : b0 + BC]
    fp = fpp[b0 : b0 + BC]
    tg = tags[b0 : b0 + BC]
    fp4 = fp.reshape(BC, TT, K, K)          # [b, t, n(next), p(prev)]

    # fwd tiles: [i, b, t, (n, kk)] = fp4[b, t, n, i*KB+kk]
    fwd = fp4[:, 0:H].reshape(BC, H, K, NS, KB).transpose(3, 0, 1, 2, 4)
    fppF = np.ascontiguousarray(fwd.reshape(NS, BC, H, K * KB)).astype(ml_dtypes.bfloat16)

    # bwd slot s holds matrix t = TT-2-s, transposed:
    # [i, b, s, (p, kk)] = fp4[b, t, n=i*KB+kk, p]
    bwd_t = fp4[:, H : TT - 1][:, ::-1]     # [b, s, n, p]
    bwd = bwd_t.reshape(BC, HB, NS, KB, K).transpose(2, 0, 1, 4, 3)
    fppB = np.ascontiguousarray(bwd.reshape(NS, BC, HB, K * KB)).astype(ml_dtypes.bfloat16)

    # winit sliced to the (i, b) partition layout: [i*32+b, kk]
    winit = np.ascontiguousarray(
        fp4[:, TT - 1, STOP, :].reshape(BC, NS, KB).transpose(1, 0, 2).reshape(P, KB),
        np.float32)

    ftp2 = np.zeros((H, UROW, BC), np.float32)
    ftp2[1:, 0:K, :] = fe[:, 0 : H - 1].transpose(1, 2, 0)
    ftp2[:, 32 : 32 + K, :] = fe[:, TT - 1 : H - 1 : -1].transpose(1, 2, 0)
    eflast = np.ascontiguousarray(fe[:, H - 1, :].T, np.float32)

    tgi = np.asarray(tg, np.int64)
    te = np.concatenate([np.full((BC, 1), START, np.int64), tgi,
                         np.full((BC, 1), STOP, np.int64)], axis=1)
    nxt, prv = te[:, 1:], te[:, :-1]
    b_ = np.arange(BC)[:, None]
    t_ = np.arange(TT)[None, :]
    gvals = np.zeros((BC, 3 * TT + 4), np.float32)
    gvals[:, 0 : TT + 1] = transitions[nxt, prv]
    gvals[:, TT + 1 : 2 * TT + 1] = np.take_along_axis(
        fe, tgi[:, :, None], axis=2)[..., 0]
    gvals[:, 2 * TT + 1 : 3 * TT + 1] = fp4[b_, np.minimum(t_, TT - 2),
                                            nxt[:, 0:TT], prv[:, 0:TT]]
    gvals[:, 3 * TT] = fp4[np.arange(BC), TT - 1, STOP, tgi[:, -1]]
    gvals[:, 3 * TT - 1] = fp4[np.arange(BC), TT - 2, nxt[:, TT - 2], prv[:, TT - 2]]

    selDN = np.zeros((UROW, K), np.float32)
    for m in range(K):
        selDN[32 + m, m] = 1.0
    sel32 = np.zeros((P, 32), np.float32)
    sel4 = np.zeros((P, P), np.float32)
    for i in range(NS):
        for b in range(32):
            sel32[i * 32 + b, b] = 1.0
            for i2 in range(NS):
                sel4[i * 32 + b, i2 * 32 + b] = 1.0
    selpack = np.zeros((P, 32 + P + K), np.float32)
    selpack[:, 0:32] = sel32
    selpack[:, 32 : 32 + P] = sel4
    selpack[0:UROW, 32 + P :] = selDN
    transPK = np.zeros((UROW, K), np.float32)
    transPK[0:K, :] = transitions.T
    transPK[32 : 32 + K, :] = transitions

    return {
        "fppF": fppF,
        "fppB": fppB,
        "winit": winit,
        "ftp2": ftp2.astype(ml_dtypes.bfloat16),
        "eflast": eflast,
        "transPK": transPK,
        "gvals": gvals,
        "selpack": selpack.astype(ml_dtypes.bfloat16),
    }


_NC_CACHE = {}
_BUILD_KW = {"PAIRCOPY": "none"}


def get_nc(BC, TT, **kw):
    if not kw:
        kw = dict(_BUILD_KW)
    key = (BC, TT, tuple(sorted(kw.items())))
    if key not in _NC_CACHE:
        _NC_CACHE[key] = build_kernel(BC=BC, TT=TT, **kw)
    return _NC_CACHE[key]


def kernel(feats, feats_pp, transitions, tags):
    feats = np.asarray(feats, np.float32)
    feats_pp = np.asarray(feats_pp, np.float32)
    transitions = np.asarray(transitions, np.float32)
    tags_np = np.asarray(tags)

    BC = B // NCORES
    nc = get_nc(BC, T, **_BUILD_KW)
    in_maps = [
        prep_core_inputs(feats, feats_pp, transitions, tags_np, c * BC, BC, T)
        for c in range(NCORES)
    ]
    r = run_bass_kernel_spmd(nc, in_maps, list(range(NCORES)))
    out = np.concatenate([r.results[c]["nll"] for c in range(NCORES)])
    return out.astype(np.float32)


# revision 3
# speedup vs baseline: 1.0026x; 1.0026x over previous
"""DTranNER CRF loss kernel for Trainium2 (8 NeuronCores, data-parallel over batch).

v2 redesign vs baseline:
  * 128-partition layout: (i, b) with i = 4 k-blocks of 6 states, b = 32
    sentences  ->  DVE mult [128,144] (2x bf16), ACT exp at full width.
  * bf16 HBM stream (host casts fpp/feats to bf16): halves DMA bytes.
  * No renorm anywhere: exp pre-scales (CP pairwise, CU+CT unary with
    CU+CT = 1+ln 24) keep per-step growth ~1; fp32/bf16 exponent range
    absorbs the +-4-sigma drift of 256-step half-chains.
  * PSUM->SBUF state copies split across ACT (fwd) / DVE (bwd) to balance
    engine busy; unary state stays in PSUM (read directly by the DVE mult).
"""

import numpy as np
import ml_dtypes
from contextlib import ExitStack

import concourse.bass as bass
import concourse.bacc as bacc
import concourse.tile as tile
from concourse import mybir
from concourse.bass_utils import run_bass_kernel_spmd

FP = mybir.dt.float32
BF = mybir.dt.bfloat16

B, T, K = 256, 512, 24
START, STOP = 22, 23
NCORES = 8
NS, KB = 4, 6          # K = NS*KB k-block split
P = NS * 32            # 128 partitions (i-major: p = i*32 + b)
UROW = 64

CP = 3.678            # pairwise exp pre-scale = ln(24)+0.5 (zero mean drift)
CU = 2.0              # unary emission exp pre-scale
CT = 2.0374           # unary transition pre-scale (CU+CT = measured natural rate)

AF = mybir.ActivationFunctionType
ALU = mybir.AluOpType
AX = mybir.AxisListType


def build_kernel(BC=32, TT=512, TC=20, CHOP=2, FIRST=8, SBUFS=3, BIGB=2, EBIGB=3,
                 FCOPY="dve", BCOPY="dve", JOINT=0, PAIRCOPY="dve", UCOPY="psum",
                 PSB=2, PSBF=None):
    assert BC == 32
    NF2 = K * KB       # 144
    H = TT // 2        # fwd pairwise steps (matrices t = 0..H-1)
    HB = TT - 1 - H    # bwd steps (matrices t = TT-2..H, transposed)
    SL = H             # unary slots

    nc = bacc.Bacc("TRN2", target_bir_lowering=False)
    fppF = nc.dram_tensor("fppF", [NS, BC, H, NF2], BF, kind="ExternalInput")
    fppB = nc.dram_tensor("fppB", [NS, BC, HB, NF2], BF, kind="ExternalInput")
    winit = nc.dram_tensor("winit", [P, KB], FP, kind="ExternalInput")
    ftp2 = nc.dram_tensor("ftp2", [SL, UROW, BC], BF, kind="ExternalInput")
    eflast = nc.dram_tensor("eflast", [K, BC], FP, kind="ExternalInput")
    transPK = nc.dram_tensor("transPK", [UROW, K], FP, kind="ExternalInput")
    gvals = nc.dram_tensor("gvals", [BC, 3 * TT + 4], FP, kind="ExternalInput")
    selpack = nc.dram_tensor("selpack", [P, 32 + P + K], BF, kind="ExternalInput")
    nll = nc.dram_tensor("nll", [BC], FP, kind="ExternalOutput")

    with tile.TileContext(nc) as tc, ExitStack() as ctx:
        sb = ctx.enter_context(tc.tile_pool(name="sb", bufs=SBUFS))
        big = ctx.enter_context(tc.tile_pool(name="big", bufs=BIGB))
        ebig = ctx.enter_context(tc.tile_pool(name="ebig", bufs=EBIGB))
        per = ctx.enter_context(tc.tile_pool(name="per", bufs=1))
        psF = ctx.enter_context(tc.tile_pool(name="psF", bufs=(PSBF or PSB), space="PSUM"))
        psB = ctx.enter_context(tc.tile_pool(name="psB", bufs=PSB, space="PSUM"))
        psU = ctx.enter_context(tc.tile_pool(name="psU", bufs=2, space="PSUM"))
        ps1 = ctx.enter_context(tc.tile_pool(name="ps1", bufs=1, space="PSUM"))

        # ---------------- constants ----------------
        cpb = per.tile([128, 1], FP, tag="cpb", name="cpb")
        nc.vector.memset(cpb[:], -CP)
        cub = per.tile([128, 1], FP, tag="cub", name="cub")
        nc.vector.memset(cub[:], -CU)
        ctb = per.tile([128, 1], FP, tag="ctb", name="ctb")
        nc.vector.memset(ctb[:], -CT)

        selpack_sb = per.tile([P, 32 + P + K], BF, tag="selpack", name="selpack_sb")
        nc.sync.dma_start(out=selpack_sb[:], in_=selpack[:])
        sel32_sb = selpack_sb[:, 0:32]
        sel4_sb = selpack_sb[:, 32 : 32 + P]
        selDN_sb = selpack_sb[0:UROW, 32 + P : 32 + P + K]

        # unary stationary weights (block matrix): uw = exp(transPK - CT)
        uwst = per.tile([UROW, K], FP, tag="uwst", name="uwst")
        nc.sync.dma_start(out=uwst[:], in_=transPK[:])
        uw = per.tile([UROW, UROW], BF, tag="uw", name="uw")
        nc.vector.memset(uw[:], 0.0)
        nc.scalar.activation(out=uw[0:K, 0:K], in_=uwst[0:K, :], func=AF.Exp,
                             bias=ctb[0:K, :])
        nc.scalar.activation(out=uw[32 : 32 + K, 32 : 32 + K], in_=uwst[32 : 32 + K, :],
                             func=AF.Exp, bias=ctb[0:K, :])

        # ---------------- unary Ef table ----------------
        eft = per.tile([UROW, SL * BC], BF, tag="eft", name="eft")
        nchunk = 8
        cs2 = SL // nchunk
        cstep = cs2 * BC
        src = ftp2[:, :, :].rearrange("s r j -> r s j")

        def load_eft_chunk(c):
            ftile = big.tile([UROW, cstep], BF, tag="ftp_in", name="ftile_u")
            nc.sync.dma_start(
                out=ftile[:].rearrange("p (s j) -> p s j", j=BC),
                in_=src[:, c * cs2 : (c + 1) * cs2, :],
            )
            nc.scalar.activation(
                out=eft[:, c * cstep : (c + 1) * cstep], in_=ftile[:],
                func=AF.Exp, bias=cub[0:UROW, :],
            )

        load_eft_chunk(0)

        # ---------------- state init ----------------
        # fwd pairwise state: e_START one-hot. START=22 -> i=3, kk=4.
        uf0 = per.tile([P, KB], BF, tag="uf0", name="uf0")
        nc.vector.memset(uf0[:], 0.0)
        nc.vector.memset(uf0[96:128, 4:5], 1.0)

        # bwd pairwise init: exp(fpp[b, T-1, STOP, :] - CP), sliced (i b) kk
        wf = sb.tile([P, KB], FP, tag="wf", name="wf")
        nc.sync.dma_start(out=wf[:], in_=winit[:, :])
        ub0 = per.tile([P, KB], BF, tag="ub0", name="ub0")
        nc.scalar.activation(out=ub0[:], in_=wf[:], func=AF.Exp, bias=cpb[0:P, :])

        # unary state [UROW, BC]
        us0 = per.tile([UROW, BC], BF, tag="us0", name="us0")
        nc.vector.memset(us0[:], 0.0)
        row1 = sb.tile([1, BC], BF, tag="row1", name="row1")
        nc.vector.memset(row1[:], 1.0)
        nc.sync.dma_start(out=us0[START : START + 1, :], in_=row1[:])
        tstop = sb.tile([UROW, 1], FP, tag="tstop", name="tstop")
        nc.sync.dma_start(
            out=tstop[32 : 32 + K, :],
            in_=transPK[32 + STOP : 32 + STOP + 1, :].rearrange("o k -> k o"),
        )
        tstop_e = sb.tile([UROW, 1], BF, tag="tstop_e", name="tstop_e")
        nc.scalar.activation(out=tstop_e[32 : 32 + K, :], in_=tstop[32 : 32 + K, :], func=AF.Exp)
        nc.vector.tensor_copy(
            out=us0[32 : 32 + K, :], in_=tstop_e[32 : 32 + K, :].broadcast_to([K, BC])
        )

        # ---------------- helpers ----------------
        gv = per.tile([BC, 3 * TT + 4], FP, tag="gv", name="gv")
        sc_early = per.tile([BC, 1], FP, tag="sc_early", name="sc_early")

        def pair_mm(eX, m, st, ups, c0, tag, ptile):
            """Pairwise chain step: DVE mult + 24 accumulating PE matmuls into
            ups[:, c0:c0+KB].  `st` is a 2D [P, KB] AP (SBUF or PSUM).
            prod goes into ptile[:, m*NF2:(m+1)*NF2] (per-chunk tile, subtile
            deps -> no per-step WAW sem waits on DVE)."""
            e3 = eX[:, m * NF2 : (m + 1) * NF2].rearrange("q (a b) -> q a b", a=K)
            p3 = ptile[:, m * NF2 : (m + 1) * NF2].rearrange("q (a b) -> q a b", a=K)
            ub = st.unsqueeze(1).broadcast_to([P, K, KB])
            nc.vector.tensor_tensor(out=p3, in0=e3, in1=ub, op=ALU.mult)
            for ip in range(NS):
                tp = (0, ip * 32)
                for kk in range(KB):
                    rhs = p3[:, ip * KB : (ip + 1) * KB, kk]
                    nc.tensor.matmul(
                        out=ups[ip * 32 : (ip + 1) * 32, c0 : c0 + KB],
                        lhsT=sel32_sb, rhs=rhs,
                        start=(kk == 0), stop=(kk == KB - 1),
                        tile_position=tp,
                    )

        def chain_copy(ups, eng, tag):
            ns_ = sb.tile([P, KB], BF, tag=f"ns{tag}", name=f"ns{tag}")
            if eng == "act":
                nc.scalar.activation(out=ns_[:], in_=ups[:, 0:KB], func=AF.Copy)
            else:
                nc.vector.tensor_copy(out=ns_[:], in_=ups[:, 0:KB])
            return ns_

        def chain_copy_joint(ups, hadB):
            w = 2 * KB if hadB else KB
            ns_ = sb.tile([P, 2 * KB], BF, tag="nsJ", name="nsJ")
            nc.vector.tensor_copy(out=ns_[:, 0:w], in_=ups[:, 0:w])
            return ns_

        # ---------------- main streamed loop ----------------
        def exp_chunks(nt):
            cs = (nt + CHOP - 1) // CHOP if CHOP else nt
            return [(a, min(a + cs, nt)) for a in range(0, nt, cs)]

        plan = [0]
        t_acc = min(FIRST, H) if FIRST else min(TC, H)
        while t_acc < H:
            plan.append(t_acc)
            t_acc += min(TC, H - t_acc)
        stF, stB = uf0[:, :], ub0[:, :]
        pendF = pendB = pendJ = pendU = None
        stU = us0
        nU = 0
        for it, t0 in enumerate(plan):
            if 1 <= it <= nchunk - 1:
                load_eft_chunk(it)
            if it == nchunk:
                nc.sync.dma_start(out=gv[:], in_=gvals[:])
                nc.vector.tensor_reduce(out=sc_early[:], in_=gv[:], axis=AX.X, op=ALU.add)
            t_next = plan[it + 1] if it + 1 < len(plan) else H
            ntF = t_next - t0
            ntB = max(0, min(t_next, HB) - t0)
            ftileF = big.tile([P, TC * NF2], BF, tag="ftileF", name="ftileF")
            for c0, c1 in exp_chunks(ntF):
                nc.sync.dma_start(
                    out=ftileF[:, c0 * NF2 : c1 * NF2],
                    in_=fppF[:, :, t0 + c0 : t0 + c1, :].rearrange("i b t f -> (i b) (t f)"),
                )
            eF = ebig.tile([P, TC * NF2], BF, tag="eF", name="eF")
            for c0, c1 in exp_chunks(ntF):
                nc.scalar.activation(
                    out=eF[:, c0 * NF2 : c1 * NF2], in_=ftileF[:, c0 * NF2 : c1 * NF2],
                    func=AF.Exp, bias=cpb[0:P, :],
                )
            if ntB > 0:
                ftileB = big.tile([P, TC * NF2], BF, tag="ftileB", name="ftileB")
                for c0, c1 in exp_chunks(ntB):
                    nc.sync.dma_start(
                        out=ftileB[:, c0 * NF2 : c1 * NF2],
                        in_=fppB[:, :, t0 + c0 : t0 + c1, :].rearrange("i b t f -> (i b) (t f)"),
                    )
                eB = ebig.tile([P, TC * NF2], BF, tag="eB", name="eB")
                for c0, c1 in exp_chunks(ntB):
                    nc.scalar.activation(
                        out=eB[:, c0 * NF2 : c1 * NF2], in_=ftileB[:, c0 * NF2 : c1 * NF2],
                        func=AF.Exp, bias=cpb[0:P, :],
                    )

            prodF_t = big.tile([P, TC * NF2], BF, tag="prodF", name="prodF_t")
            prodB_t = big.tile([P, TC * NF2], BF, tag="prodB", name="prodB_t")
            usm_t = big.tile([UROW, TC * BC], BF, tag="usm_t", name="usm_t")
            for m in range(ntF):
                # ---- unary slot ----
                g = nU
                ef_sl = eft[:, g * BC : (g + 1) * BC]
                if pendU is not None:
                    stU_sb = sb.tile([UROW, BC], BF, tag="stU", name="stU_sb")
                    nc.scalar.activation(out=stU_sb[:], in_=pendU[:], func=AF.Copy)
                    stU = stU_sb
                    pendU = None
                usm = usm_t[:, m * BC : (m + 1) * BC]
                nc.vector.tensor_tensor(out=usm, in0=stU[:], in1=ef_sl, op=ALU.mult)
                nU += 1
                vu_ps = psU.tile([UROW, BC], FP, tag="vu", name="vu_ps")
                nc.tensor.matmul(out=vu_ps[:], lhsT=uw[:], rhs=usm, start=True, stop=True)
                if UCOPY == "act":
                    pendU = vu_ps
                else:
                    stU = vu_ps

                has_b = m < ntB
                if JOINT:
                    # one PSUM tile for both chains, one joint copy
                    if pendJ is not None:
                        stJ = chain_copy_joint(pendJ[0], pendJ[1])
                        stF = stJ[:][:, 0:KB]
                        if pendJ[1]:
                            stB = stJ[:][:, KB : 2 * KB]
                        pendJ = None
                    upsJ = psF.tile([P, 2 * KB], FP, tag="upsJ", name="upsJ")
                    pair_mm(eF, m, stF, upsJ, 0, "F", prodF_t)
                    if has_b:
                        pair_mm(eB, m, stB, upsJ, KB, "B", prodB_t)
                    pendJ = (upsJ, has_b)
                elif PAIRCOPY == "none":
                    # mults read PSUM state directly; no copies
                    upsF = psF.tile([P, KB], FP, tag="upsF", name="upsF")
                    pair_mm(eF, m, stF, upsF, 0, "F", prodF_t)
                    stF = upsF[:, :]
                    if has_b:
                        upsB = psB.tile([P, KB], FP, tag="upsB", name="upsB")
                        pair_mm(eB, m, stB, upsB, KB * 0, "B", prodB_t)
                        stB = upsB[:, :]
                else:
                    if pendF is not None:
                        stF = chain_copy(pendF, FCOPY, "F")[:, :]
                        pendF = None
                    upsF = psF.tile([P, KB], FP, tag="upsF", name="upsF")
                    pair_mm(eF, m, stF, upsF, 0, "F", prodF_t)
                    pendF = upsF

                    if pendB is not None:
                        stB = chain_copy(pendB, BCOPY, "B")[:, :]
                        pendB = None
                    if has_b:
                        upsB = psB.tile([P, KB], FP, tag="upsB", name="upsB")
                        pair_mm(eB, m, stB, upsB, 0, "B", prodB_t)
                        pendB = upsB

        # ---------------- flush final deferred copies ----------------
        if pendJ is not None:
            stJ = chain_copy_joint(pendJ[0], pendJ[1])
            stF = stJ[:][:, 0:KB]
            if pendJ[1]:
                stB = stJ[:][:, KB : 2 * KB]
            pendJ = None
        if pendF is not None:
            stF = chain_copy(pendF, FCOPY, "F")[:, :]
            pendF = None
        if pendB is not None:
            stB = chain_copy(pendB, BCOPY, "B")[:, :]
            pendB = None
        if pendU is not None:
            stU_sb = sb.tile([UROW, BC], BF, tag="stU", name="stU_sb")
            nc.scalar.activation(out=stU_sb[:], in_=pendU[:], func=AF.Copy)
            stU = stU_sb
            pendU = None
        if PAIRCOPY == "none":
            # final states live in PSUM; the meet mult may read only one
            # PSUM operand, so land stB in SBUF first
            stB_sb = sb.tile([P, KB], BF, tag="nsB", name="stB_fin")
            nc.vector.tensor_copy(out=stB_sb[:], in_=stB)
            stB = stB_sb[:, :]

        # ---------------- tails ----------------
        # pairwise meet: q[b] = sum_k uF[b,k]*uB[b,k]
        qm = sb.tile([P, KB], BF, tag="qm", name="qm")
        nc.vector.tensor_tensor(out=qm[:], in0=stF, in1=stB, op=ALU.mult)
        qr = sb.tile([P, 1], BF, tag="qr", name="qr")
        with nc.allow_low_precision("meet partial"):
            nc.vector.tensor_reduce(out=qr[:], in_=qm[:], axis=AX.X, op=ALU.add)
        qrep = ps1.tile([P, 1], FP, tag="pmisc", name="qrep")
        nc.tensor.matmul(out=qrep[:], lhsT=sel4_sb, rhs=qr[:], start=True, stop=True)
        lq = sb.tile([P, 1], FP, tag="lq", name="lq")
        nc.scalar.activation(out=lq[:], in_=qrep[:], func=AF.Ln)

        # unary tail: all per-b results assembled partition-major [32, 1]
        efl = sb.tile([K, BC], FP, tag="efl", name="efl")
        nc.sync.dma_start(out=efl[:], in_=eflast[:])
        efl_e = sb.tile([K, BC], BF, tag="efl_e", name="efl_e")
        nc.scalar.activation(out=efl_e[:], in_=efl[:], func=AF.Exp)
        ustail = sb.tile([UROW, BC], BF, tag="ustail", name="ustail")
        nc.scalar.activation(out=ustail[:], in_=stU[:], func=AF.Copy)
        # bring bwd rows 32..32+K down to partitions 0..K via selector matmul
        usb_ps = ps1.tile([K, BC], FP, tag="pmisc", name="usb_ps")
        nc.tensor.matmul(out=usb_ps[:], lhsT=selDN_sb, rhs=ustail[:], start=True, stop=True)
        um = sb.tile([K, BC], BF, tag="um", name="um")
        nc.vector.tensor_tensor(out=um[:], in0=ustail[0:K, :], in1=efl_e[:], op=ALU.mult)
        nc.vector.tensor_tensor(out=um[:], in0=um[:], in1=usb_ps[:], op=ALU.mult)
        ones_k = sb.tile([K, 1], BF, tag="ones_k", name="ones_k")
        nc.vector.memset(ones_k[:], 1.0)
        au_t = ps1.tile([32, 1], FP, tag="pmisc", name="au_t")
        nc.tensor.matmul(out=au_t[:], lhsT=um[:], rhs=ones_k[:], start=True, stop=True)
        lau_t = sb.tile([32, 1], FP, tag="lau_t", name="lau_t")
        nc.scalar.activation(out=lau_t[:], in_=au_t[:], func=AF.Ln)

        # score reduction + final combine (all [32, 1] partition-major)
        CONST = CP * (H + HB + 1) + (CU + CT) * (2 * SL)
        res = sb.tile([BC, 1], FP, tag="res", name="res")
        nc.vector.tensor_tensor(out=res[:], in0=lq[0:32, :], in1=lau_t[:], op=ALU.add)
        nc.vector.tensor_scalar(out=res[:], in0=res[:], scalar1=CONST, scalar2=None, op0=ALU.add)
        nc.vector.tensor_tensor(out=res[:], in0=res[:], in1=sc_early[:], op=ALU.subtract)
        nc.sync.dma_start(out=nll[:], in_=res[:].rearrange("b o -> (b o)"))

    nc.compile()
    return nc


# ======================= host-side prep =======================

def prep_core_inputs(feats, fpp, transitions, tags, b0, BC, TT):
    H = TT // 2
    HB = TT - 1 - H
    fe = feats[b0 : b0 + BC]
    fp = fpp[b0 : b0 + BC]
    tg = tags[b0 : b0 + BC]
    fp4 = fp.reshape(BC, TT, K, K)          # [b, t, n(next), p(prev)]

    # fwd tiles: [i, b, t, (n, kk)] = fp4[b, t, n, i*KB+kk]
    fwd = fp4[:, 0:H].reshape(BC, H, K, NS, KB).transpose(3, 0, 1, 2, 4)
    fppF = np.ascontiguousarray(fwd.reshape(NS, BC, H, K * KB)).astype(ml_dtypes.bfloat16)

    # bwd slot s holds matrix t = TT-2-s, transposed:
    # [i, b, s, (p, kk)] = fp4[b, t, n=i*KB+kk, p]
    bwd_t = fp4[:, H : TT - 1][:, ::-1]     # [b, s, n, p]
    bwd = bwd_t.reshape(BC, HB, NS, KB, K).transpose(2, 0, 1, 4, 3)
    fppB = np.ascontiguousarray(bwd.reshape(NS, BC, HB, K * KB)).astype(ml_dtypes.bfloat16)

    # winit sliced to the (i, b) partition layout: [i*32+b, kk]
    winit = np.ascontiguousarray(
        fp4[:, TT - 1, STOP, :].reshape(BC, NS, KB).transpose(1, 0, 2).reshape(P, KB),
        np.float32)

    ftp2 = np.zeros((H, UROW, BC), np.float32)
    ftp2[1:, 0:K, :] = fe[:, 0 : H - 1].transpose(1, 2, 0)
    ftp2[:, 32 : 32 + K, :] = fe[:, TT - 1 : H - 1 : -1].transpose(1, 2, 0)
    eflast = np.ascontiguousarray(fe[:, H - 1, :].T, np.float32)

    tgi = np.asarray(tg, np.int64)
    te = np.concatenate([np.full((BC, 1), START, np.int64), tgi,
                         np.full((BC, 1), STOP, np.int64)], axis=1)
    nxt, prv = te[:, 1:], te[:, :-1]
    b_ = np.arange(BC)[:, None]
    t_ = np.arange(TT)[None, :]
    gvals = np.zeros((BC, 3 * TT + 4), np.float32)
    gvals[:, 0 : TT + 1] = transitions[nxt, prv]
    gvals[:, TT + 1 : 2 * TT + 1] = np.take_along_axis(
        fe, tgi[:, :, None], axis=2)[..., 0]
    gvals[:, 2 * TT + 1 : 3 * TT + 1] = fp4[b_, np.minimum(t_, TT - 2),
                                            nxt[:, 0:TT], prv[:, 0:TT]]
    gvals[:, 3 * TT] = fp4[np.arange(BC), TT - 1, STOP, tgi[:, -1]]
    gvals[:, 3 * TT - 1] = fp4[np.arange(BC), TT - 2, nxt[:, TT - 2], prv[:, TT - 2]]

    selDN = np.zeros((UROW, K), np.float32)
    for m in range(K):
        selDN[32 + m, m] = 1.0
    sel32 = np.zeros((P, 32), np.float32)
    sel4 = np.zeros((P, P), np.float32)
    for i in range(NS):
        for b in range(32):
            sel32[i * 32 + b, b] = 1.0
            for i2 in range(NS):
                sel4[i * 32 + b, i2 * 32 + b] = 1.0
    selpack = np.zeros((P, 32 + P + K), np.float32)
    selpack[:, 0:32] = sel32
    selpack[:, 32 : 32 + P] = sel4
    selpack[0:UROW, 32 + P :] = selDN
    transPK = np.zeros((UROW, K), np.float32)
    transPK[0:K, :] = transitions.T
    transPK[32 : 32 + K, :] = transitions

    return {
        "fppF": fppF,
        "fppB": fppB,
        "winit": winit,
        "ftp2": ftp2.astype(ml_dtypes.bfloat16),
        "eflast": eflast,
        "transPK": transPK,
        "gvals": gvals,
        "selpack": selpack.astype(ml_dtypes.bfloat16),
    }


_NC_CACHE = {}
_BUILD_KW = {"PAIRCOPY": "none", "FIRST": 12}


def get_nc(BC, TT, **kw):
    if not kw:
        kw = dict(_BUILD_KW)
    key = (BC, TT, tuple(sorted(kw.items())))
    if key not in _NC_CACHE:
        _NC_CACHE[key] = build_kernel(BC=BC, TT=TT, **kw)
    return _NC_CACHE[key]


def kernel(feats, feats_pp, transitions, tags):
    feats = np.asarray(feats, np.float32)
    feats_pp = np.asarray(feats_pp, np.float32)
    transitions = np.asarray(transitions, np.float32)
    tags_np = np.asarray(tags)

    BC = B // NCORES
    nc = get_nc(BC, T, **_BUILD_KW)
    in_maps = [
        prep_core_inputs(feats, feats_pp, transitions, tags_np, c * BC, BC, T)
        for c in range(NCORES)
    ]
    r = run_bass_kernel_spmd(nc, in_maps, list(range(NCORES)))
    out = np.concatenate([r.results[c]["nll"] for c in range(NCORES)])
    return out.astype(np.float32)
